# revision 3
# baseline (speedup 1.0000x reference)
"""Bahdanau attention kernel for 8 Trainium2 NeuronCores.

Problem shapes (hardcoded): hidden [2, 32, 1024], encoder_outputs [32, 2048, 1024],
Wq/Wk [1024, 1024], bq/bk/wv [1024], bv scalar. Output [32, 1, 1024].

Sharding: data-parallel over batch B=32 -> 4 batches per core, weights replicated.
bv is dropped entirely (softmax is invariant to constant shifts).

Key structure (v4):
- The PE runs (almost) nothing but the K-projection: enc @ Wk.T in fp8e4 with
  MatmulPerfMode.DoubleRow (2 fp8 MACs/cell/cycle), 4 accumulating MMs of
  contraction 256 per (o-tile, s-chunk). Wk is pre-scaled by 64 on the host so
  its values sit in fp8's normal range; the inverse folds into the tanh scale.
- The q-projection also runs fp8 DoubleRow (Wq x64, hidden x8 prescale; the
  1/512 descale folds into the bias-add).
- Everything else moved OFF the PE:
  * scores wv-contraction: 8 chained scalar_tensor_tensor ops on DVE (4x mode)
    produce th3[p, s] = sum_i wv[128i+p] * tanh[128i+p, s].
  * cross-partition sum of th3: GPSIMD partition_all_reduce (output is the
    scores row replicated on all 128 partitions - the broadcast for free).
  * exp: ACT engine on the [128, 512] replicated tile, accum_out -> softmax sum.
  * attn @ enc einsum: 8 scalar_tensor_tensor ops with accum_out on DVE over
    the SAME transposed-layout enc tiles (bf16 copy), accumulated per h-column
    pair (p, i); no PE matmuls, no transposes.
- enc ships twice (fp8 [h, s] tiles for the projection; bf16 same layout for
  the einsum); all layout/dtype prep is host-side in make_in_maps.
- scores never materialize with a max-shift (|scores| <= sum|wv| <= 32, safe
  in fp32/bf16 exp); the final [128, 8] accumulator is scaled by 1/sum and
  written transposed (outT); the host undoes the transpose.
"""

from contextlib import ExitStack

import numpy as np

import concourse.bacc as bacc
import concourse.bass as bass
import concourse.mybir as mybir
import concourse.tile as tile
from concourse.bass_utils import run_bass_kernel_spmd
from concourse import bass_isa

B, S, H = 32, 2048, 1024
NCORES = 8
BPC = B // NCORES  # 4 batches per core
F32 = mybir.dt.float32
BF16 = mybir.dt.bfloat16
FP8 = mybir.dt.float8e4
HT = H // 128  # 8 chunks of 128 along h or o
SC = S // 512  # 4 s-chunks of 512
KT = 4  # fp8 DoubleRow: 4 contraction steps of 256
WK_SCALE = 64.0
HID_SCALE = 8.0
Tanh = mybir.ActivationFunctionType.Tanh
Exp = mybir.ActivationFunctionType.Exp
X = mybir.AxisListType.X
DR = mybir.MatmulPerfMode.DoubleRow
Mult = mybir.AluOpType.mult
Add = mybir.AluOpType.add
RAdd = bass_isa.ReduceOp.add

ts = bass.ts


def build_program():
    nc = bacc.Bacc("TRN2", target_bir_lowering=False, debug=False)

    # enc^T fp8 tiles: encT8[b, j, p, i, s] = fp8(enc[b, 512j+s, 128i+p])
    encT8_d = nc.dram_tensor("encT8", [BPC, SC, 128, HT, 512], FP8, kind="ExternalInput")
    # enc^T bf16 tiles, same layout (einsum operand)
    encT16_d = nc.dram_tensor("encT16", [BPC, SC, 128, HT, 512], BF16, kind="ExternalInput")
    # Wk^T fp8 (x64): wkT8[p, i, c, m] = fp8(64 * Wk[128i+m, 128c+p])
    wkT8_d = nc.dram_tensor("wkT8", [128, HT, HT, 128], FP8, kind="ExternalInput")
    # Wq^T fp8 (x64): wqT8[p, t, c, n] = fp8(64 * Wq[128t+n, 128c+p])
    wqT8_d = nc.dram_tensor("wqT8", [128, HT, HT, 128], FP8, kind="ExternalInput")
    # hid^T fp8 (x8): hidT8[p, c, b] = fp8(8 * hidden[-1][b, 128c+p])
    hidT8_d = nc.dram_tensor("hidT8", [128, HT, BPC], FP8, kind="ExternalInput")
    bqkT_d = nc.dram_tensor("bqkT", [128, HT], F32, kind="ExternalInput")  # (bq+bk)^T
    wvT_d = nc.dram_tensor("wvT", [128, HT], F32, kind="ExternalInput")  # wv^T
    # transposed output: outT[b, p, i] = out[b, 128i+p]
    outT_d = nc.dram_tensor("outT", [BPC, 128, HT], F32, kind="ExternalOutput")

    with tile.TileContext(nc) as tc, ExitStack() as ctx:
        consts = ctx.enter_context(tc.tile_pool(name="consts", bufs=1))
        kp = ctx.enter_context(tc.tile_pool(name="kp", bufs=5, space="PSUM"))
        pq_p = ctx.enter_context(tc.tile_pool(name="pqp", bufs=1, space="PSUM"))
        encT_p = ctx.enter_context(tc.tile_pool(name="encT", bufs=5))  # 512KB/slot
        enc16_p = ctx.enter_context(tc.tile_pool(name="enc16", bufs=5))  # 1MB/slot
        eT_p = ctx.enter_context(tc.tile_pool(name="eT", bufs=2))  # 1MB/slot
        sc_p = ctx.enter_context(tc.tile_pool(name="sc", bufs=2))
        batch = ctx.enter_context(tc.tile_pool(name="batch", bufs=1))

        # ---- staging helpers (chunk granular, plain HWDGE DMAs) ----
        def load_enc_chunk(b, j):
            eT8 = encT_p.tile([128, HT, 512], FP8, tag="encT8")
            nc.sync.dma_start(eT8[:], encT8_d[b, j])
            e16 = enc16_p.tile([128, HT, 512], BF16, tag="enc16")
            nc.sync.dma_start(e16[:], encT16_d[b, j])
            return eT8, e16

        # ---- weights + small consts; q-projection inputs land first so the
        # PE can run qproj while the first enc chunk + wkT8 stream in. ----
        wqT8 = consts.tile([128, HT, HT, 128], FP8, tag="wqT8")
        nc.sync.dma_start(wqT8[:], wqT8_d[:])
        hidT8 = consts.tile([128, HT, BPC], FP8, tag="hidT8")
        nc.scalar.dma_start(hidT8[:], hidT8_d[:])
        bqkT = consts.tile([128, HT], F32, tag="bqkT")
        nc.scalar.dma_start(bqkT[:], bqkT_d[:])
        wvT = consts.tile([128, HT], F32, tag="wvT")
        nc.scalar.dma_start(wvT[:], wvT_d[:])

        wkT8 = consts.tile([128, HT, HT, 128], FP8, tag="wkT8")
        nc.sync.dma_start(wkT8[:], wkT8_d[:])
        staged = {}
        staged[(0, 0)] = load_enc_chunk(0, 0)
        staged[(0, 1)] = load_enc_chunk(0, 1)
        staged[(0, 2)] = load_enc_chunk(0, 2)

        # ---- q^T + bq + bk on the PE in fp8 DR while enc chunk 0 streams:
        # qkb[o(part), o-chunk t, b] = q/512 + bq + bk ----
        qkb = consts.tile([128, HT, BPC], F32, tag="qkb")
        for t in range(HT):
            pq = pq_p.tile([128, BPC], F32, tag="pq")
            for kt in range(KT):
                nc.tensor.matmul(
                    pq[:],
                    wqT8[:, t, ts(kt, 2), :],
                    hidT8[:, ts(kt, 2), :],
                    start=(kt == 0),
                    stop=(kt == KT - 1),
                    perf_mode=DR,
                )
            nc.vector.tensor_scalar(
                qkb[:, t, :], pq[:], 1.0 / (WK_SCALE * HID_SCALE),
                bqkT[:, t : t + 1], op0=Mult, op1=Add,
            )

        staged[(0, 3)] = load_enc_chunk(0, 3)

        def kproj_mm_chain(i, eT8):
            pk = kp.tile([128, 512], F32, tag="kp", name="pk")
            for kt in range(KT):
                nc.tensor.matmul(
                    pk[:],
                    wkT8[:, i, ts(kt, 2), :],
                    eT8[:, ts(kt, 2), :],
                    start=(kt == 0),
                    stop=(kt == KT - 1),
                    perf_mode=DR,
                )
            return pk

        # ---- per-chunk tail pieces (pipelined one chunk behind kproj) ----
        def tail_allreduce(p):
            # cross-partition sum of th3 -> scores row replicated on all
            # 128 partitions (GPSIMD; doubles as the broadcast for exp).
            th3r = sc_p.tile([128, 512], BF16, tag="th3r")
            nc.gpsimd.partition_all_reduce(th3r[:], p["th3"][:], 128, RAdd)
            p["th3r"] = th3r

        def tail_exp(p):
            attn = sc_p.tile([128, 512], BF16, tag="attn")
            nc.scalar.activation(
                attn[:], p["th3r"][:], Exp,
                accum_out=p["ssum4"][:, p["j"] : p["j"] + 1],
            )
            p["attn"] = attn

        def tail_einsum(p):
            # attn-weighted sum of enc rows: accJ[p_, i] = sum_s attn[s] *
            # enc[512j+s, 128i+p_], all on DVE (4x mode stt with accum_out).
            accJ = sc_p.tile([128, HT], F32, tag="accJ")
            scr = sc_p.tile([128, 512], BF16, tag="scr")
            for i in range(HT):
                nc.vector.scalar_tensor_tensor(
                    scr[:], p["e16"][:, i, :], 1.0, p["attn"][:],
                    op0=Mult, op1=Mult, accum_out=accJ[:, i : i + 1],
                )
            b = p["b"]
            if p["j"] == 0:
                nc.vector.tensor_copy(acc8s[b][:], accJ[:])
            else:
                nc.vector.tensor_add(acc8s[b][:], acc8s[b][:], accJ[:])

        def finish_batch(b):
            ssum = batch.tile([128, 1], F32, tag="ssum", bufs=2)
            nc.vector.reduce_sum(ssum[:], ssum4s[b][:], axis=X)
            inv = batch.tile([128, 1], F32, tag="inv", bufs=2)
            nc.vector.reciprocal(inv[:], ssum[:])
            outF = batch.tile([128, HT], F32, tag="outF", bufs=2)
            nc.vector.tensor_scalar_mul(outF[:], acc8s[b][:], inv[:, 0:1])
            nc.sync.dma_start(outT_d[b][:], outF[:])

        # per-batch accumulators
        acc8s = [
            batch.tile([128, HT], F32, tag=f"acc8_{b}", name=f"acc8_{b}")
            for b in range(BPC)
        ]
        ssum4s = [
            batch.tile([128, SC], F32, tag=f"ssum4_{b}", name=f"ssum4_{b}")
            for b in range(BPC)
        ]

        pending = None
        for b in range(BPC):
            for j in range(SC):
                # stage chunk (b+1, j) while computing chunk (b, j)
                if b + 1 < BPC:
                    staged[(b + 1, j)] = load_enc_chunk(b + 1, j)
                eT8, e16 = staged.pop((b, j))
                p = pending

                if p is not None:
                    tail_allreduce(p)

                # K^T tiles (fp8 DoubleRow) + fused bias/tanh (undoes the x64
                # prescale) -> eT_j[o(part), o-chunk i, s(512)], then the DVE
                # scores chain: th = th + wv_i * tanh_i (ping-pong tiles).
                eT_j = eT_p.tile([128, HT, 512], BF16, tag="eTj")
                thA = sc_p.tile([128, 512], BF16, tag="thA")
                thB = sc_p.tile([128, 512], BF16, tag="thB")
                cur, nxt = thA, thB
                for i in range(HT):
                    pk = kproj_mm_chain(i, eT8)
                    nc.scalar.activation(
                        eT_j[:, i, :], pk[:], Tanh,
                        bias=qkb[:, i, b : b + 1], scale=1.0 / WK_SCALE,
                    )
                    if i == 0:
                        nc.vector.tensor_scalar_mul(
                            cur[:], eT_j[:, 0, :], wvT[:, 0:1]
                        )
                    else:
                        nc.vector.scalar_tensor_tensor(
                            nxt[:], eT_j[:, i, :], wvT[:, i : i + 1], cur[:],
                            op0=Mult, op1=Add,
                        )
                        cur, nxt = nxt, cur
                    if i == 2 and p is not None:
                        tail_exp(p)

                if p is not None:
                    tail_einsum(p)
                    if p["j"] == SC - 1:
                        finish_batch(p["b"])

                pending = dict(b=b, j=j, th3=cur, e16=e16, ssum4=ssum4s[b])

        # flush the final chunk
        p = pending
        tail_allreduce(p)
        tail_exp(p)
        tail_einsum(p)
        finish_batch(p["b"])

    nc.compile()
    return nc


_CACHED_NC = None


def _get_nc():
    global _CACHED_NC
    if _CACHED_NC is None:
        _CACHED_NC = build_program()
    return _CACHED_NC


_F8 = mybir.dt.np(FP8)
_BF = mybir.dt.np(BF16)


def make_in_maps(hidden, encoder_outputs, Wq, bq, Wk, bk, wv):
    """Host-side shard + layout prep (all compute FLOPs stay on device)."""
    hid_last = np.asarray(hidden, np.float32)[-1]  # [32, H]
    enc = np.asarray(encoder_outputs, np.float32)
    Wq = np.asarray(Wq, np.float32)
    Wk = np.asarray(Wk, np.float32)
    bqkT = np.ascontiguousarray(
        (np.asarray(bq, np.float32) + np.asarray(bk, np.float32)).reshape(HT, 128).T
    )
    wvT = np.ascontiguousarray(np.asarray(wv, np.float32).reshape(HT, 128).T)

    # enc^T tiles: [B, SC, 128(p), HT(i), 512(s)]
    encT = np.ascontiguousarray(
        np.clip(enc, -240, 240)
        .reshape(B, SC, 512, HT, 128)
        .transpose(0, 1, 4, 3, 2)
    )
    encT8 = encT.astype(_F8)
    encT16 = encT.astype(_BF)

    # W^T fp8 with x64 prescale: [128(p), HT(i), HT(c), 128(m)]
    def wT8(W):
        return np.ascontiguousarray(
            np.clip(W * WK_SCALE, -240, 240)
            .reshape(HT, 128, HT, 128)
            .transpose(3, 0, 2, 1)
        ).astype(_F8)

    wkT8 = wT8(Wk)
    wqT8 = wT8(Wq)

    in_maps = []
    for c in range(NCORES):
        sl = slice(c * BPC, (c + 1) * BPC)
        # hid^T fp8 x8: [128(p), HT(c), BPC(b)]
        hidT8 = np.ascontiguousarray(
            np.clip(hid_last[sl] * HID_SCALE, -240, 240)
            .reshape(BPC, HT, 128)
            .transpose(2, 1, 0)
        ).astype(_F8)
        in_maps.append(
            {
                "encT8": np.ascontiguousarray(encT8[sl]),
                "encT16": np.ascontiguousarray(encT16[sl]),
                "wkT8": wkT8,
                "wqT8": wqT8,
                "hidT8": hidT8,
                "bqkT": bqkT,
                "wvT": wvT,
            }
        )
    return in_maps


def run(inputs, trace=False):
    """Run on hardware; returns (output [32,1,1024], BassKernelResults)."""
    nc = _get_nc()
    in_maps = make_in_maps(
        inputs["hidden"],
        inputs["encoder_outputs"],
        inputs["Wq"],
        inputs["bq"],
        inputs["Wk"],
        inputs["bk"],
        inputs["wv"],
    )
    res = run_bass_kernel_spmd(nc, in_maps, list(range(NCORES)), trace=trace)
    # outT[b, p, i] -> out[b, 128i+p]
    outs = []
    for c in range(NCORES):
        outT = res.results[c]["outT"]  # [BPC, 128, HT]
        outs.append(outT.transpose(0, 2, 1).reshape(BPC, 1, H))
    out = np.concatenate(outs, axis=0)
    return out.astype(np.float32), res


def kernel(hidden, encoder_outputs, Wq, bq, Wk, bk, wv, bv):
    out, _ = run(
        {
            "hidden": hidden,
            "encoder_outputs": encoder_outputs,
            "Wq": Wq,
            "bq": bq,
            "Wk": Wk,
            "bk": bk,
            "wv": wv,
        }
    )
    return out


# revision 4
# speedup vs baseline: 1.3164x; 1.3164x over previous
"""Bahdanau attention kernel for 8 Trainium2 NeuronCores.

Problem shapes (hardcoded): hidden [2, 32, 1024], encoder_outputs [32, 2048, 1024],
Wq/Wk [1024, 1024], bq/bk/wv [1024], bv scalar. Output [32, 1, 1024].

Sharding: data-parallel over batch B=32 -> 4 batches per core, weights replicated.
bv is dropped entirely (softmax is invariant to constant shifts).

Key structure (v5):
- The PE runs the K-projection (enc @ Wk.T) in fp8e4 DoubleRow (2 fp8 MACs per
  cell per cycle): 4 accumulating MMs of contraction 256 per (o-tile, s-chunk).
  Wk is pre-scaled x64 on the host; the inverse folds into the tanh scale.
- The q-projection also runs fp8 DoubleRow (Wq x64, hidden x8; the 1/512
  descale folds into the bias-add on DVE).
- scores = wv . tanh(q+k): computed on the PE as 8 chained matmuls with
  RANK-1 weights (wvmat[:, i, :] = wv[128i+p] replicated over columns), so
  the [128, 512] PSUM result is the scores row replicated on all 128
  partitions - the partition-broadcast for the einsum comes for free, and no
  transposes are needed anywhere. These MMs interleave between kproj chains
  one step behind the tanh producer, so the PE never waits on the ACT engine.
- exp runs on ACT straight from the scores PSUM (no max-shift needed:
  |scores| <= sum|wv| <= 32), accum_out collects the softmax denominator.
- attn @ enc einsum: 8 scalar_tensor_tensor ops with accum_out on DVE over a
  transposed bf16 copy of enc (same [h-part, s] tiling as the fp8 tiles);
  per-chunk partials land in acc4[:, i, j] and a single reduce at batch end
  folds the chunks. The final [128, 8] column is scaled by the reciprocal
  softmax sum and written transposed (outT); the host undoes the transpose.
- enc ships twice (fp8 + bf16, both pre-transposed); all layout/dtype prep
  is host-side in make_in_maps; staging is chunk-granular DMA in deep rings.
"""

from contextlib import ExitStack

import numpy as np

import concourse.bacc as bacc
import concourse.bass as bass
import concourse.mybir as mybir
import concourse.tile as tile
from concourse.bass_utils import run_bass_kernel_spmd

B, S, H = 32, 2048, 1024
NCORES = 8
BPC = B // NCORES  # 4 batches per core
F32 = mybir.dt.float32
BF16 = mybir.dt.bfloat16
FP8 = mybir.dt.float8e4
HT = H // 128  # 8 chunks of 128 along h or o
SC = S // 512  # 4 s-chunks of 512
KT = 4  # fp8 DoubleRow: 4 contraction steps of 256
WK_SCALE = 64.0
HID_SCALE = 8.0
Tanh = mybir.ActivationFunctionType.Tanh
Exp = mybir.ActivationFunctionType.Exp
X = mybir.AxisListType.X
DR = mybir.MatmulPerfMode.DoubleRow
Mult = mybir.AluOpType.mult
Add = mybir.AluOpType.add

ts = bass.ts


def build_program():
    nc = bacc.Bacc("TRN2", target_bir_lowering=False, debug=False)

    # enc^T fp8 tiles: encT8[b, j, p, i, s] = fp8(enc[b, 512j+s, 128i+p])
    encT8_d = nc.dram_tensor("encT8", [BPC, SC, 128, HT, 512], FP8, kind="ExternalInput")
    # enc^T bf16 tiles, same layout (einsum operand)
    encT16_d = nc.dram_tensor("encT16", [BPC, SC, 128, HT, 512], BF16, kind="ExternalInput")
    # Wk^T fp8 (x64): wkT8[p, i, c, m] = fp8(64 * Wk[128i+m, 128c+p])
    wkT8_d = nc.dram_tensor("wkT8", [128, HT, HT, 128], FP8, kind="ExternalInput")
    # Wq^T fp8 (x64): wqT8[p, t, c, n] = fp8(64 * Wq[128t+n, 128c+p])
    wqT8_d = nc.dram_tensor("wqT8", [128, HT, HT, 128], FP8, kind="ExternalInput")
    # hid^T fp8 (x8): hidT8[p, c, b] = fp8(8 * hidden[-1][b, 128c+p])
    hidT8_d = nc.dram_tensor("hidT8", [128, HT, BPC], FP8, kind="ExternalInput")
    bqkT_d = nc.dram_tensor("bqkT", [128, HT], F32, kind="ExternalInput")  # (bq+bk)^T
    # rank-1 scores weights: wvmat[p, i, m] = wv[128i+p] (replicated over m)
    wvmat_d = nc.dram_tensor("wvmat", [128, HT, 128], BF16, kind="ExternalInput")
    # transposed output: outT[b, p, i] = out[b, 128i+p]
    outT_d = nc.dram_tensor("outT", [BPC, 128, HT], F32, kind="ExternalOutput")

    with tile.TileContext(nc) as tc, ExitStack() as ctx:
        consts = ctx.enter_context(tc.tile_pool(name="consts", bufs=1))
        kp = ctx.enter_context(tc.tile_pool(name="kp", bufs=5, space="PSUM"))
        ps_p = ctx.enter_context(tc.tile_pool(name="psp", bufs=2, space="PSUM"))
        pq_p = ctx.enter_context(tc.tile_pool(name="pqp", bufs=1, space="PSUM"))
        encT_p = ctx.enter_context(tc.tile_pool(name="encT", bufs=5))  # 512KB/slot
        enc16_p = ctx.enter_context(tc.tile_pool(name="enc16", bufs=5))  # 1MB/slot
        eT_p = ctx.enter_context(tc.tile_pool(name="eT", bufs=2))  # 1MB/slot
        sc_p = ctx.enter_context(tc.tile_pool(name="sc", bufs=2))
        batch = ctx.enter_context(tc.tile_pool(name="batch", bufs=1))

        # ---- staging helpers (chunk granular, plain HWDGE DMAs) ----
        def load_enc_chunk(b, j):
            eT8 = encT_p.tile([128, HT, 512], FP8, tag="encT8")
            nc.sync.dma_start(eT8[:], encT8_d[b, j])
            e16 = enc16_p.tile([128, HT, 512], BF16, tag="enc16")
            nc.sync.dma_start(e16[:], encT16_d[b, j])
            return eT8, e16

        # ---- weights + small consts; q-projection inputs land first so the
        # PE can run qproj while the first enc chunk + wkT8 stream in. ----
        wqT8 = consts.tile([128, HT, HT, 128], FP8, tag="wqT8")
        nc.sync.dma_start(wqT8[:], wqT8_d[:])
        hidT8 = consts.tile([128, HT, BPC], FP8, tag="hidT8")
        nc.scalar.dma_start(hidT8[:], hidT8_d[:])
        bqkT = consts.tile([128, HT], F32, tag="bqkT")
        nc.scalar.dma_start(bqkT[:], bqkT_d[:])
        wvmat = consts.tile([128, HT, 128], BF16, tag="wvmat")
        nc.scalar.dma_start(wvmat[:], wvmat_d[:])

        wkT8 = consts.tile([128, HT, HT, 128], FP8, tag="wkT8")
        nc.sync.dma_start(wkT8[:], wkT8_d[:])
        staged = {}
        staged[(0, 0)] = load_enc_chunk(0, 0)
        staged[(0, 1)] = load_enc_chunk(0, 1)
        staged[(0, 2)] = load_enc_chunk(0, 2)

        # ---- q^T + bq + bk on the PE in fp8 DR while enc chunk 0 streams:
        # qkb[o(part), o-chunk t, b] = q/512 + bq + bk ----
        qkb = consts.tile([128, HT, BPC], F32, tag="qkb")
        for t in range(HT):
            pq = pq_p.tile([128, BPC], F32, tag="pq")
            for kt in range(KT):
                nc.tensor.matmul(
                    pq[:],
                    wqT8[:, t, ts(kt, 2), :],
                    hidT8[:, ts(kt, 2), :],
                    start=(kt == 0),
                    stop=(kt == KT - 1),
                    perf_mode=DR,
                )
            nc.vector.tensor_scalar(
                qkb[:, t, :], pq[:], 1.0 / (WK_SCALE * HID_SCALE),
                bqkT[:, t : t + 1], op0=Mult, op1=Add,
            )

        staged[(0, 3)] = load_enc_chunk(0, 3)

        def kproj_mm_chain(i, eT8):
            pk = kp.tile([128, 512], F32, tag="kp", name="pk")
            for kt in range(KT):
                nc.tensor.matmul(
                    pk[:],
                    wkT8[:, i, ts(kt, 2), :],
                    eT8[:, ts(kt, 2), :],
                    start=(kt == 0),
                    stop=(kt == KT - 1),
                    perf_mode=DR,
                )
            return pk

        def scores_mm(c, i):
            # ps[m, s] += sum_p wv[128i+p] * tanh[p, i, s]; M=128 replicated
            # rows = the scores row broadcast to every partition.
            nc.tensor.matmul(
                c["ps"][:],
                wvmat[:, i, :],
                c["eT_j"][:, i, :],
                start=(i == 0),
                stop=(i == HT - 1),
            )

        def tail_exp(p):
            attn = sc_p.tile([128, 512], BF16, tag="attn")
            nc.scalar.activation(
                attn[:], p["ps"][:], Exp,
                accum_out=ssum4s[p["b"]][:, p["j"] : p["j"] + 1],
            )
            p["attn"] = attn

        def tail_einsum(p):
            # attn-weighted sum of enc rows on DVE: acc4[p_, i, j] =
            # sum_s attn[s] * enc[512j+s, 128i+p_].
            b, j = p["b"], p["j"]
            for i in range(HT):
                nc.vector.scalar_tensor_tensor(
                    scr[:], p["e16"][:, i, :], 1.0, p["attn"][:],
                    op0=Mult, op1=Mult,
                    accum_out=acc4s[b][:, i, j : j + 1],
                )

        def finish_batch(b):
            accH = batch.tile([128, HT], F32, tag="accH", bufs=2)
            nc.vector.reduce_sum(accH[:], acc4s[b][:], axis=X)
            ssum = batch.tile([128, 1], F32, tag="ssum", bufs=2)
            nc.vector.reduce_sum(ssum[:], ssum4s[b][:], axis=X)
            inv = batch.tile([128, 1], F32, tag="inv", bufs=2)
            nc.vector.reciprocal(inv[:], ssum[:])
            outF = batch.tile([128, HT], F32, tag="outF", bufs=2)
            nc.vector.tensor_scalar_mul(outF[:], accH[:], inv[:, 0:1])
            nc.sync.dma_start(outT_d[b][:], outF[:])

        # per-batch accumulators + einsum scratch sink
        acc4s = [
            batch.tile([128, HT, SC], F32, tag=f"acc4_{b}", name=f"acc4_{b}")
            for b in range(BPC)
        ]
        ssum4s = [
            batch.tile([128, SC], F32, tag=f"ssum4_{b}", name=f"ssum4_{b}")
            for b in range(BPC)
        ]
        scr = consts.tile([128, 512], BF16, tag="scr")

        pending = None
        for b in range(BPC):
            for j in range(SC):
                # stage chunk (b+1, j) while computing chunk (b, j)
                if b + 1 < BPC:
                    staged[(b + 1, j)] = load_enc_chunk(b + 1, j)
                eT8, e16 = staged.pop((b, j))
                p = pending
                cur = dict(
                    b=b, j=j, e16=e16,
                    eT_j=eT_p.tile([128, HT, 512], BF16, tag="eTj", name="eTj"),
                    ps=ps_p.tile([128, 512], F32, tag="ps", name="ps"),
                )
                for i in range(HT):
                    pk = kproj_mm_chain(i, eT8)
                    nc.scalar.activation(
                        cur["eT_j"][:, i, :], pk[:], Tanh,
                        bias=qkb[:, i, b : b + 1], scale=1.0 / WK_SCALE,
                    )
                    if i == 0 and p is not None:
                        # prev chunk's last scores MM + its tail
                        scores_mm(p, HT - 1)
                        tail_exp(p)
                        tail_einsum(p)
                        if p["j"] == SC - 1:
                            finish_batch(p["b"])
                    if i >= 1:
                        # scores MM i-1 of THIS chunk (tanh_{i-1} is done by
                        # the time chain_i finishes streaming)
                        scores_mm(cur, i - 1)
                pending = cur

        # flush the final chunk
        p = pending
        scores_mm(p, HT - 1)
        tail_exp(p)
        tail_einsum(p)
        finish_batch(p["b"])

    nc.compile()
    return nc


_CACHED_NC = None


def _get_nc():
    global _CACHED_NC
    if _CACHED_NC is None:
        _CACHED_NC = build_program()
    return _CACHED_NC


_F8 = mybir.dt.np(FP8)
_BF = mybir.dt.np(BF16)


def make_in_maps(hidden, encoder_outputs, Wq, bq, Wk, bk, wv):
    """Host-side shard + layout prep (all compute FLOPs stay on device)."""
    hid_last = np.asarray(hidden, np.float32)[-1]  # [32, H]
    enc = np.asarray(encoder_outputs, np.float32)
    Wq = np.asarray(Wq, np.float32)
    Wk = np.asarray(Wk, np.float32)
    wv = np.asarray(wv, np.float32)
    bqkT = np.ascontiguousarray(
        (np.asarray(bq, np.float32) + np.asarray(bk, np.float32)).reshape(HT, 128).T
    )
    # rank-1 scores weights: [128(p), HT(i), 128(m)] = wv[128i+p]
    wvmat = np.ascontiguousarray(
        np.broadcast_to(wv.reshape(HT, 128).T[:, :, None], (128, HT, 128))
    ).astype(_BF)

    # enc^T tiles: [B, SC, 128(p), HT(i), 512(s)]
    encT = np.ascontiguousarray(
        np.clip(enc, -240, 240)
        .reshape(B, SC, 512, HT, 128)
        .transpose(0, 1, 4, 3, 2)
    )
    encT8 = encT.astype(_F8)
    encT16 = encT.astype(_BF)

    # W^T fp8 with x64 prescale: [128(p), HT(i), HT(c), 128(m)]
    def wT8(W):
        return np.ascontiguousarray(
            np.clip(W * WK_SCALE, -240, 240)
            .reshape(HT, 128, HT, 128)
            .transpose(3, 0, 2, 1)
        ).astype(_F8)

    wkT8 = wT8(Wk)
    wqT8 = wT8(Wq)

    in_maps = []
    for c in range(NCORES):
        sl = slice(c * BPC, (c + 1) * BPC)
        # hid^T fp8 x8: [128(p), HT(c), BPC(b)]
        hidT8 = np.ascontiguousarray(
            np.clip(hid_last[sl] * HID_SCALE, -240, 240)
            .reshape(BPC, HT, 128)
            .transpose(2, 1, 0)
        ).astype(_F8)
        in_maps.append(
            {
                "encT8": np.ascontiguousarray(encT8[sl]),
                "encT16": np.ascontiguousarray(encT16[sl]),
                "wkT8": wkT8,
                "wqT8": wqT8,
                "hidT8": hidT8,
                "bqkT": bqkT,
                "wvmat": wvmat,
            }
        )
    return in_maps


def run(inputs, trace=False):
    """Run on hardware; returns (output [32,1,1024], BassKernelResults)."""
    nc = _get_nc()
    in_maps = make_in_maps(
        inputs["hidden"],
        inputs["encoder_outputs"],
        inputs["Wq"],
        inputs["bq"],
        inputs["Wk"],
        inputs["bk"],
        inputs["wv"],
    )
    res = run_bass_kernel_spmd(nc, in_maps, list(range(NCORES)), trace=trace)
    # outT[b, p, i] -> out[b, 128i+p]
    outs = []
    for c in range(NCORES):
        outT = res.results[c]["outT"]  # [BPC, 128, HT]
        outs.append(outT.transpose(0, 2, 1).reshape(BPC, 1, H))
    out = np.concatenate(outs, axis=0)
    return out.astype(np.float32), res


def kernel(hidden, encoder_outputs, Wq, bq, Wk, bk, wv, bv):
    out, _ = run(
        {
            "hidden": hidden,
            "encoder_outputs": encoder_outputs,
            "Wq": Wq,
            "bq": bq,
            "Wk": Wk,
            "bk": bk,
            "wv": wv,
        }
    )
    return out
